# revision 29
# baseline (speedup 1.0000x reference)
"""HGCN forward on 8 TRN2 NeuronCores — restructured for throughput.

Strategy (graph/data parallel):
- Nodes padded to 100352 = 8*12544, sharded 12544/core (98 tiles of 128).
- Per-layer: matmul with an extra weight column (W^T u_b) so the mobius
  u_b dot-product falls out of the matmul; per-node scalar chains
  (expmap/mobius/logmap coefficients) are computed BATCHED on [128, 98]
  tiles once per phase instead of per-tile [128,1] ops.
- Gather table (logmap features) stored fp16: halves HBM gather traffic,
  AllGather bytes, and runs the segment-sum matmuls at fp16 PE rate.
- hyp_agg: edges sorted by destination tile; ONE batched indirect DMA
  gathers all chunks for a 7-tile block (amortizes the ~1us SWDGE fixed
  cost); one-hot*weight matrices built on DVE/Pool; dst-tile aggregates
  accumulate on the TensorEngine in PSUM.
"""
import os, sys, types
import numpy as np

sys.path.insert(0, "/opt/trn_rl_repo")

# NTFF profile hook shim (antenv.axon_hooks is absent in this image).
if "antenv.axon_hooks" not in sys.modules:
    _m = types.ModuleType("antenv.axon_hooks")
    _hh = [None]
    _m.set_axon_ntff_profile_hook = lambda h: _hh.__setitem__(0, h)
    _m.get_axon_ntff_profile_hook = lambda: _hh[0]
    sys.modules["antenv.axon_hooks"] = _m
    try:
        from trn_agent_boot.trn_boot import _ntff_profile_via_ctypes
        _m.set_axon_ntff_profile_hook(_ntff_profile_via_ctypes("/opt/axon/libaxon_pjrt.so"))
    except Exception:
        pass

import concourse.bass as bass
import concourse.tile as tile
from concourse import bacc, mybir
import concourse.bass_utils as _bu
_bu.upload_artifacts = lambda d: "local://skipped"
from concourse.bass_utils import run_bass_kernel_spmd
from concourse.library_config import mlp as _mlp_lib
from contextlib import ExitStack

F = np.float32
F16 = np.float16
EPS = 1e-7
MIN = 1e-15
NC = 8
P = 128
DT = mybir.dt.float32
DT16 = mybir.dt.float16
SKS = [float(np.sqrt(3.0)), float(np.sqrt(2.0)), 1.0]


def _host_ub(b, c):
    # u_b = logmap0(proj(expmap0(proj_tan0(b), c), c), c), faithful f32.
    K = F(1.0 / c)
    sK = F(np.sqrt(K))
    y = b[1:].astype(F)
    yn = max(np.sqrt((y * y).sum(dtype=F)), F(MIN))
    th = min(yn / sK, F(15.0))
    sh = F(np.sinh(th))
    ch = F(np.cosh(th))
    hb_s = sK * sh * y / yn
    hb0 = F(np.sqrt(max(K + (hb_s * hb_s).sum(dtype=F), F(EPS))))
    thh = max(hb0 / sK, F(1.0 + EPS))
    ac = F(np.log(thh + np.sqrt(thh * thh - 1)))
    ybn = max(F(np.sqrt((hb_s * hb_s).sum(dtype=F))), F(MIN))
    u_s = sK * ac * hb_s / ybn
    out = np.zeros(b.shape[0], F)
    out[1:] = u_s
    return out


def _build(T, Kcws, NPAD, B):
    """One SPMD program for all 8 cores. T tiles/core, Kcws chunks per
    src-window per tile, B tiles per gather/load block."""
    S = T * P
    NB = T // B
    KcE = int(sum(Kcws))
    WIN = 32768
    win_rows = [min(WIN, NPAD - w * WIN) for w in range(len(Kcws))]
    col_base = [0]
    for kc in Kcws:
        col_base.append(col_base[-1] + kc)
    IW = B * KcE * 8   # int16 idx cols per block
    DE = 129   # mv cols (128) + dub col
    DE3 = 65   # out_d (64) + dub col
    OD = 64
    nc = bacc.Bacc("TRN2", target_bir_lowering=False, debug=False, num_devices=NC)

    xpT_d = nc.dram_tensor("xpT", [NB, P, B * P], DT, kind="ExternalInput")
    idx_d = nc.dram_tensor("idx", [NB, P, IW], mybir.dt.int16, kind="ExternalInput")
    meta_d = nc.dram_tensor("meta", [NB, P, 2 * B * KcE], DT16, kind="ExternalInput")
    c32_d = nc.dram_tensor("c32", [P, DE], DT, kind="ExternalInput")
    c16_d = nc.dram_tensor("c16", [P, DE + DE3 + 4 * P + OD + KcE * P], DT16, kind="ExternalInput")
    out_d_t = nc.dram_tensor("out", [S, OD], DT, kind="ExternalOutput")

    xt1_sh = nc.dram_tensor("xt1_sh", [S, P], DT16)
    xt1_full = nc.dram_tensor("xt1_full", [NPAD, P], DT16, addr_space="Shared")
    xt2_sh = nc.dram_tensor("xt2_sh", [S, P], DT16)
    xt2_full = nc.dram_tensor("xt2_full", [NPAD, P], DT16, addr_space="Shared")

    A = mybir.AluOpType
    AF = mybir.ActivationFunctionType

    with tile.TileContext(nc) as tc, ExitStack() as ctx:
        cp = ctx.enter_context(tc.tile_pool(name="consts", bufs=1))
        xbp = ctx.enter_context(tc.tile_pool(name="xb", bufs=2))
        gp = ctx.enter_context(tc.tile_pool(name="gath", bufs=3))
        ibp = ctx.enter_context(tc.tile_pool(name="ib", bufs=2))
        mbp = ctx.enter_context(tc.tile_pool(name="mb", bufs=2))
        mtp = ctx.enter_context(tc.tile_pool(name="mt", bufs=2))
        rp = ctx.enter_context(tc.tile_pool(name="rp", bufs=3))
        rtp = ctx.enter_context(tc.tile_pool(name="rt", bufs=3))
        mvp = ctx.enter_context(tc.tile_pool(name="mv", bufs=1))
        btp = ctx.enter_context(tc.tile_pool(name="bt", bufs=1))
        stp = ctx.enter_context(tc.tile_pool(name="st", bufs=2))
        xop = ctx.enter_context(tc.tile_pool(name="xo", bufs=3))
        pag = ctx.enter_context(tc.tile_pool(name="pag", bufs=2, space="PSUM"))
        pmv = ctx.enter_context(tc.tile_pool(name="pmv", bufs=2, space="PSUM"))
        ptr = ctx.enter_context(tc.tile_pool(name="ptr", bufs=2, space="PSUM"))

        nc.gpsimd.load_library(_mlp_lib)
        c32 = cp.tile([P, DE], DT, name="c32t")
        nc.sync.dma_start(out=c32[:], in_=c32_d[:])
        c16 = cp.tile([P, DE + DE3 + 4 * P + OD + KcE * P], DT16, name="c16t")
        nc.sync.dma_start(out=c16[:], in_=c16_d[:])
        W1p = c32[:, 0:DE]
        o = 0
        W2p = c16[:, o:o + DE]; o += DE
        Wlp = c16[:, o:o + DE3]; o += DE3
        IOTA = c16[:, o:o + P]; o += P
        IDN = c16[:, o:o + P]; o += P
        UB1S = c16[:, o:o + P]; o += P
        UB2S = c16[:, o:o + P]; o += P
        UBLS = c16[:, o:o + OD]; o += OD
        IOTAW = c16[:, o:o + KcE * P].rearrange("p (c e) -> p c e", e=P); o += KcE * P

        # UBN2 constants are provided via closure at prep time
        UBNS = _build.ubns

        def bt_(nm):
            return btp.tile([P, T], DT, tag=nm, name=nm)

        def ts(o_, i_, s1, s2, op0, op1=None, eng=None):
            e = eng or nc.vector
            if op1 is None:
                e.tensor_scalar(o_[:], i_[:], s1, None, op0)
            else:
                e.tensor_scalar(o_[:], i_[:], s1, s2, op0, op1)

        def chain(MN2ap, DUBap, k_in, UBN2, AN2t=None, Y42t=None, k_ai=None,
                  k_ao=None, final=False):
            sk = SKS[k_in]; ik = 1.0 / sk; K = sk * sk
            if AN2t is not None:
                ski = SKS[k_ai]; iki = 1.0 / ski
                sko = SKS[k_ao]; iko = 1.0 / sko
                anr = bt_("anr"); nc.scalar.sqrt(anr[:], AN2t[:])
                anc = bt_("anc"); ts(anc, anr, MIN, None, A.max)
                th3 = bt_("th3"); ts(th3, anc, iki, 15.0, A.mult, A.min)
                ran = bt_("ran"); nc.vector.reciprocal(ran[:], anc[:])
                h3a = bt_("h3a"); nc.vector.tensor_tensor(h3a[:], th3[:], ran[:], A.mult)
                h3 = bt_("h3"); ts(h3, h3a, ski, None, A.mult)
                h3q = bt_("h3q"); nc.vector.tensor_tensor(h3q[:], h3[:], h3[:], A.mult)
                y42 = bt_("y42"); nc.vector.tensor_tensor(y42[:], Y42t[:], h3q[:], A.mult)
                y4r = bt_("y4r"); nc.scalar.sqrt(y4r[:], y42[:])
                y4c = bt_("y4c"); ts(y4c, y4r, MIN, None, A.max)
                th4 = bt_("th4"); ts(th4, y4c, iko, 15.0, A.mult, A.min)
                r4 = bt_("r4"); nc.vector.reciprocal(r4[:], y4c[:])
                m5a = bt_("m5a"); nc.vector.tensor_tensor(m5a[:], th4[:], r4[:], A.mult)
                m5 = bt_("m5"); ts(m5, m5a, sko, None, A.mult)
                sin = bt_("sin"); nc.vector.tensor_tensor(sin[:], h3[:], m5[:], A.mult)
                sin2 = bt_("sin2"); nc.vector.tensor_tensor(sin2[:], sin[:], sin[:], A.mult)
                mn2 = bt_("mn2"); nc.vector.tensor_tensor(mn2[:], MN2ap[:], sin2[:], A.mult)
                dub = bt_("dub"); nc.vector.tensor_tensor(dub[:], DUBap, sin[:], A.mult)
                mn2ap = mn2[:]; dub_ap = dub[:]
            else:
                sin = None
                mn2ap = MN2ap[:]; dub_ap = DUBap
            mnr = bt_("mnr"); nc.scalar.sqrt(mnr[:], mn2ap)
            mnc = bt_("mnc"); ts(mnc, mnr, MIN, None, A.max)
            thc = bt_("thc"); ts(thc, mnc, ik, 15.0, A.mult, A.min)
            ea = bt_("ea"); nc.scalar.activation(ea[:], thc[:], AF.Exp)
            eb = bt_("eb"); nc.scalar.activation(eb[:], thc[:], AF.Exp, scale=-1.0)
            sh2 = bt_("sh2"); nc.vector.tensor_tensor(sh2[:], ea[:], eb[:], A.subtract)
            ch2 = bt_("ch2"); nc.vector.tensor_tensor(ch2[:], ea[:], eb[:], A.add)
            rmn = bt_("rmn"); nc.vector.reciprocal(rmn[:], mnc[:])
            g1a = bt_("g1a"); nc.vector.tensor_tensor(g1a[:], sh2[:], rmn[:], A.mult)
            g1 = bt_("g1"); ts(g1, g1a, 0.5 * sk, None, A.mult)
            x0v = bt_("x0v"); ts(x0v, ch2, 0.5 * sk, None, A.mult)
            yna = bt_("yna"); nc.vector.tensor_tensor(yna[:], g1[:], mnc[:], A.mult)
            yn = bt_("yn"); ts(yn, yna, MIN, None, A.max)
            ryn = bt_("ryn"); nc.vector.reciprocal(ryn[:], yn[:])
            d1 = bt_("d1"); nc.vector.tensor_tensor(d1[:], g1[:], dub_ap, A.mult)
            ala = bt_("ala"); nc.vector.tensor_tensor(ala[:], d1[:], ryn[:], A.mult)
            alpha = bt_("alpha"); ts(alpha, ala, ik, None, A.mult)
            skx = bt_("skx"); ts(skx, x0v, sk, -1.0, A.subtract, A.mult)
            t2 = bt_("t2"); nc.vector.tensor_tensor(t2[:], alpha[:], skx[:], A.mult)
            scal1 = bt_("scal1"); nc.vector.tensor_tensor(scal1[:], t2[:], ryn[:], A.mult)
            gg = bt_("gg"); nc.vector.tensor_tensor(gg[:], scal1[:], g1[:], A.mult)
            gm = bt_("gm"); nc.vector.tensor_tensor(gm[:], g1[:], mn2ap, A.mult)
            u1 = bt_("u1"); nc.vector.tensor_tensor(u1[:], gg[:], gm[:], A.mult)
            ux = bt_("ux"); nc.vector.tensor_tensor(ux[:], d1[:], u1[:], A.subtract)
            rx0 = bt_("rx0"); nc.vector.reciprocal(rx0[:], x0v[:])
            v0 = bt_("v0"); nc.vector.tensor_tensor(v0[:], ux[:], rx0[:], A.mult)
            w1 = bt_("w1"); nc.vector.tensor_tensor(w1[:], gg[:], mn2ap, A.mult)
            # w3 = 2*dub - w1
            d2 = bt_("d2"); nc.vector.tensor_scalar(d2[:], dub_ap, 2.0, None, A.mult)
            w3 = bt_("w3"); nc.vector.tensor_tensor(w3[:], d2[:], w1[:], A.subtract)
            mdpa = bt_("mdpa"); nc.vector.tensor_tensor(mdpa[:], gg[:], w3[:], A.mult)
            mdp = bt_("mdp"); ts(mdp, mdpa, -1.0, UBN2, A.mult, A.add)
            v0q = bt_("v0q"); nc.vector.tensor_tensor(v0q[:], v0[:], v0[:], A.mult)
            md = bt_("md"); nc.vector.tensor_tensor(md[:], mdp[:], v0q[:], A.subtract)
            mdc = bt_("mdc"); ts(mdc, md, EPS, None, A.max)
            nur = bt_("nur"); nc.scalar.sqrt(nur[:], mdc[:])
            th2 = bt_("th2"); ts(th2, nur, 1e6, ik, A.min, A.mult)
            th2m = bt_("th2m"); ts(th2m, th2, MIN, None, A.max)
            th2c = bt_("th2c"); ts(th2c, th2m, 15.0, None, A.min)
            ea2 = bt_("ea2"); nc.scalar.activation(ea2[:], th2c[:], AF.Exp)
            eb2 = bt_("eb2"); nc.scalar.activation(eb2[:], th2c[:], AF.Exp, scale=-1.0)
            sh22 = bt_("sh22"); nc.vector.tensor_tensor(sh22[:], ea2[:], eb2[:], A.subtract)
            ch22 = bt_("ch22"); nc.vector.tensor_tensor(ch22[:], ea2[:], eb2[:], A.add)
            rt2 = bt_("rt2"); nc.vector.reciprocal(rt2[:], th2m[:])
            s2a = bt_("s2a"); nc.vector.tensor_tensor(s2a[:], sh22[:], rt2[:], A.mult)
            s2 = bt_("s2"); ts(s2, s2a, 0.5, None, A.mult)
            ss = bt_("ss"); nc.vector.tensor_tensor(ss[:], s2[:], scal1[:], A.mult)
            cc = bt_("cc"); ts(cc, ch22, 0.5, None, A.mult)
            Aa = bt_("Aa"); nc.vector.tensor_tensor(Aa[:], cc[:], ss[:], A.subtract)
            Av = bt_("Av"); nc.vector.tensor_tensor(Av[:], Aa[:], g1[:], A.mult)
            A2 = bt_("A2"); nc.vector.tensor_tensor(A2[:], Av[:], Av[:], A.mult)
            tA = bt_("tA"); nc.vector.tensor_tensor(tA[:], A2[:], mn2ap, A.mult)
            ABv = bt_("ABv"); nc.vector.tensor_tensor(ABv[:], Av[:], s2[:], A.mult)
            tABa = bt_("tABa")
            nc.vector.tensor_tensor(tABa[:], ABv[:], dub_ap, A.mult)
            tAB = bt_("tAB"); ts(tAB, tABa, 2.0, None, A.mult)
            tBa = bt_("tBa"); nc.vector.tensor_tensor(tBa[:], s2[:], s2[:], A.mult)
            tB = bt_("tB"); ts(tB, tBa, UBN2, None, A.mult)
            l2a = bt_("l2a"); nc.vector.tensor_tensor(l2a[:], tA[:], tAB[:], A.add)
            ln2 = bt_("ln2"); nc.vector.tensor_tensor(ln2[:], l2a[:], tB[:], A.add)
            lnk = bt_("lnk"); ts(lnk, ln2, K, None, A.add)
            L0 = bt_("L0"); nc.scalar.sqrt(L0[:], lnk[:])
            if final:
                SA = bt_("SA")
                nc.vector.tensor_tensor(SA[:], Av[:], sin[:], A.mult)
                return SA, s2, L0
            thL = bt_("thL"); ts(thL, L0, ik, 1.0 + EPS, A.mult, A.max)
            tq = bt_("tq"); nc.vector.tensor_tensor(tq[:], thL[:], thL[:], A.mult)
            tqm = bt_("tqm"); ts(tqm, tq, -1.0, None, A.add)
            sq = bt_("sq"); nc.scalar.sqrt(sq[:], tqm[:])
            ai = bt_("ai"); nc.vector.tensor_tensor(ai[:], thL[:], sq[:], A.add)
            ac = bt_("ac"); nc.scalar.activation(ac[:], ai[:], AF.Ln)
            yr2 = bt_("yr2"); nc.scalar.sqrt(yr2[:], ln2[:])
            yc2 = bt_("yc2"); ts(yc2, yr2, MIN, None, A.max)
            ry2 = bt_("ry2"); nc.vector.reciprocal(ry2[:], yc2[:])
            fLa = bt_("fLa"); nc.vector.tensor_tensor(fLa[:], ac[:], ry2[:], A.mult)
            fL = bt_("fL"); ts(fL, fLa, sk, None, A.mult)
            sAa = bt_("sAa"); nc.vector.tensor_tensor(sAa[:], fL[:], Av[:], A.mult)
            SA = bt_("SA")
            if sin is not None:
                nc.vector.tensor_tensor(SA[:], sAa[:], sin[:], A.mult)
            else:
                nc.vector.tensor_copy(SA[:], sAa[:])
            SB = bt_("SB"); nc.vector.tensor_tensor(SB[:], fL[:], s2[:], A.mult)
            return SA, SB

        # ================= Phase A =================
        MN2 = btp.tile([P, T], DT, tag="MN2", name="MN2a")
        MV = mvp.tile([P, T * DE], DT16, tag="MV", name="MVa")
        for nb in range(NB):
            xblk = xbp.tile([P, B * P], DT, name="xblk")
            nc.sync.dma_start(out=xblk[:], in_=xpT_d[nb])
            for b in range(B):
                t = nb * B + b
                mv = pmv.tile([P, DE], DT, space="PSUM", tag="mv", name="mvp")
                nc.tensor.matmul(mv[:], lhsT=xblk[:, b * P:(b + 1) * P],
                                 rhs=W1p, start=True, stop=True)
                scr = stp.tile([P, P - 1], DT, tag="s32", name="scr")
                nc.scalar.activation(scr[:], mv[:, 1:P], AF.Square,
                                     accum_out=MN2[:, t:t + 1])
                nc.scalar.copy(MV[:, t * DE:(t + 1) * DE], mv[:])
        dubA = MV[:].rearrange("p (t d) -> p t d", d=DE)[:, :, DE - 1:DE].squeeze(2)
        SA, SB = chain(MN2, dubA, 0, UBNS[0])
        for t in range(T):
            xa = xop.tile([P, P], DT16, tag="xa", name="xa")
            nc.scalar.activation(xa[:], MV[:, t * DE:t * DE + P], AF.Copy,
                                 scale=SA[:, t:t + 1])
            ub = xop.tile([P, P], DT16, tag="ub", name="ub")
            nc.vector.tensor_scalar(ub[:], UB1S, SB[:, t:t + 1], None, A.mult)
            xt = xop.tile([P, P], DT16, tag="xt", name="xt")
            nc.vector.tensor_tensor(xt[:], xa[:], ub[:], A.add)
            nc.sync.dma_start(out=xt1_sh[t * P:(t + 1) * P, :], in_=xt[:])
        nc.gpsimd.collective_compute("AllGather", A.bypass,
                                     replica_groups=[list(range(NC))],
                                     ins=[xt1_sh[:]], outs=[xt1_full[:]])

        # ============ Phases B & C (agg + linear) ============
        def agg_phase(table, Wp, DEx, k_ai, k_ao, UBN2, UBS, final, sh_out, full_out):
            MN2 = btp.tile([P, T], DT, tag="MN2", name="MN2x")
            AN2 = btp.tile([P, T], DT, tag="AN2", name="AN2x")
            Y42 = btp.tile([P, T], DT, tag="Y42", name="Y42x")
            MV = mvp.tile([P, T * DE], DT16, tag="MV", name="MVx")
            for nb in range(NB):
                idxb = ibp.tile([P, IW], mybir.dt.int16, name="idxb")
                nc.sync.dma_start(out=idxb[:], in_=idx_d[nb])
                metb = mbp.tile([P, 2 * B * KcE], DT16, name="metb")
                nc.sync.dma_start(out=metb[:], in_=meta_d[nb])
                G = gp.tile([P, B * KcE * P], DT16, tag="G", name="G")
                off_c = 0
                off_i = 0
                for w, kc in enumerate(Kcws):
                    n_w = B * kc * P
                    nc.gpsimd.dma_gather(
                        out_ap=G[:, off_c:off_c + n_w].rearrange(
                            "p (c e) -> p c e", e=P),
                        in_ap=table[w * WIN:w * WIN + win_rows[w], :],
                        idxs_ap=idxb[:, off_i:off_i + n_w // 16],
                        num_idxs=n_w, num_idxs_reg=n_w, elem_size=P,
                        single_packet=False)
                    off_c += n_w
                    off_i += n_w // 16
                for b in range(B):
                    t = nb * B + b
                    rels = metb[:, b * 2 * KcE:b * 2 * KcE + KcE]
                    wcol = metb[:, b * 2 * KcE + KcE:b * 2 * KcE + 2 * KcE]
                    MtA = mtp.tile([P, KcE * P], DT16, tag="MtA", name="MtA")
                    nc.vector.tensor_tensor(
                        MtA[:].rearrange("p (c e) -> p c e", e=P), IOTAW,
                        rels.unsqueeze(2).broadcast_to([P, KcE, P]), A.is_equal)
                    MtW = mtp.tile([P, KcE * P], DT16, tag="MtW", name="MtW")
                    nc.vector.tensor_tensor(
                        MtW[:].rearrange("p (c e) -> p c e", e=P),
                        MtA[:].rearrange("p (c e) -> p c e", e=P),
                        wcol.unsqueeze(2).broadcast_to([P, KcE, P]), A.mult)
                    agg = pag.tile([P, P], DT, space="PSUM", tag="agg", name="aggp")
                    for q in range(KcE):
                        # window of chunk q and its column in G
                        w = 0
                        while q >= col_base[w + 1]:
                            w += 1
                        j = q - col_base[w]
                        col = (B * col_base[w] + b * Kcws[w] + j) * P
                        nc.tensor.matmul(agg[:], lhsT=MtW[:, q * P:(q + 1) * P],
                                         rhs=G[:, col:col + P],
                                         start=(q == 0), stop=(q == KcE - 1))
                    s32a = stp.tile([P, P - 1], DT, tag="s32a", name="s32a")
                    nc.scalar.activation(s32a[:], agg[:, 1:P], AF.Square,
                                         accum_out=AN2[:, t:t + 1])
                    R = rp.tile([P, P], DT16, tag="R", name="R")
                    nc.scalar.activation(R[:], agg[:], AF.Relu)
                    s16b = stp.tile([P, P - 1], DT16, tag="s16b", name="s16b")
                    nc.scalar.activation(s16b[:], R[:, 1:P], AF.Square,
                                         accum_out=Y42[:, t:t + 1])
                    trp = ptr.tile([P, P], DT16, space="PSUM", tag="trp", name="trp")
                    nc.tensor.transpose(trp[:], R[:], IDN)
                    rT = rtp.tile([P, P], DT16, tag="rT", name="rT")
                    nc.scalar.copy(rT[:], trp[:])
                    mv = pmv.tile([P, DEx], DT, space="PSUM", tag="mv", name="mvp")
                    nc.tensor.matmul(mv[:], lhsT=rT[:], rhs=Wp, start=True, stop=True)
                    scr = stp.tile([P, P - 1], DT, tag="s32", name="scr")
                    nc.scalar.activation(scr[:, 0:DEx - 2], mv[:, 1:DEx - 1], AF.Square,
                                         accum_out=MN2[:, t:t + 1])
                    nc.scalar.copy(MV[:, t * DE:t * DE + DEx], mv[:])
            dubv = MV[:].rearrange("p (t d) -> p t d", d=DE)[:, :, DEx - 1:DEx].squeeze(2)
            if final:
                SA3, SB3, L0 = chain(MN2, dubv, k_ao, UBN2, AN2, Y42, k_ai, k_ao,
                                     final=True)
                for t in range(T):
                    o1 = xop.tile([P, OD - 1], DT, tag="o1", name="o1")
                    nc.scalar.activation(o1[:], MV[:, t * DE + 1:t * DE + OD], AF.Copy,
                                         scale=SA3[:, t:t + 1])
                    u3 = xop.tile([P, OD - 1], DT, tag="u3", name="u3")
                    nc.vector.tensor_scalar(u3[:], UBS[:, 1:OD], SB3[:, t:t + 1],
                                            None, A.mult)
                    ot = xop.tile([P, OD], DT, tag="ot", name="ot")
                    nc.vector.tensor_tensor(ot[:, 1:OD], o1[:], u3[:], A.add)
                    nc.vector.tensor_copy(ot[:, 0:1], L0[:, t:t + 1])
                    nc.sync.dma_start(out=out_d_t[t * P:(t + 1) * P, :], in_=ot[:])
            else:
                SA, SB = chain(MN2, dubv, k_ao, UBN2, AN2, Y42, k_ai, k_ao)
                for t in range(T):
                    xa = xop.tile([P, P], DT16, tag="xa", name="xa")
                    nc.scalar.activation(xa[:], MV[:, t * DE:t * DE + P], AF.Copy,
                                         scale=SA[:, t:t + 1])
                    ub = xop.tile([P, P], DT16, tag="ub", name="ub")
                    nc.vector.tensor_scalar(ub[:], UBS, SB[:, t:t + 1], None, A.mult)
                    xt = xop.tile([P, P], DT16, tag="xt", name="xt")
                    nc.vector.tensor_tensor(xt[:], xa[:], ub[:], A.add)
                    nc.sync.dma_start(out=sh_out[t * P:(t + 1) * P, :], in_=xt[:])
                nc.gpsimd.collective_compute("AllGather", A.bypass,
                                             replica_groups=[list(range(NC))],
                                             ins=[sh_out[:]], outs=[full_out[:]])

        stages = int(os.environ.get("KSTAGES", "3"))
        if stages >= 2:
            agg_phase(xt1_full, W2p, DE, 0, 1, UBNS[1], UB2S, False, xt2_sh, xt2_full)
        if stages >= 3:
            agg_phase(xt2_full, Wlp, DE3, 1, 2, UBNS[2], UBLS, True, None, None)
        if stages < 3:
            z = xop.tile([P, OD], DT, tag="z", name="z")
            nc.vector.memset(z[:], 0.0)
            for t in range(T):
                nc.sync.dma_start(out=out_d_t[t * P:(t + 1) * P, :], in_=z[:])

    nc.compile()
    return nc


def _prep(x, edge_index, edge_weight, W1, b1, W2, b2, Wl, bl, NPAD, B):
    N = x.shape[0]
    S = NPAD // NC
    T = S // P
    GT = NPAD // P
    NB = T // B
    WIN = 32768
    NW = (NPAD + WIN - 1) // WIN
    src = edge_index[0].astype(np.int64)
    dst = edge_index[1].astype(np.int64)
    w = edge_weight.astype(F)
    gt_u = dst >> 7
    wid_u = src >> 15
    # group edges by (dst tile, src window)
    order = np.lexsort((wid_u, gt_u))
    srcs, dsts, ws = src[order], dst[order], w[order]
    gt, wid = gt_u[order], wid_u[order]
    cnt_tw = np.zeros((GT, NW), np.int64)
    np.add.at(cnt_tw, (gt, wid), 1)
    Kcws = [max(1, int(np.ceil(cnt_tw[:, ww].max() / P))) for ww in range(NW)]
    KcE = int(sum(Kcws))
    CAPS = [kc * P for kc in Kcws]
    col_base = np.zeros(NW + 1, np.int64)
    col_base[1:] = np.cumsum(CAPS)
    CAP = int(col_base[-1])
    starts_tw = np.zeros(GT * NW, np.int64)
    starts_tw[1:] = np.cumsum(cnt_tw.reshape(-1))[:-1]
    starts_tw = starts_tw.reshape(GT, NW)
    pos = np.arange(len(srcs)) - starts_tw[gt, wid]
    slot = col_base[wid] + pos
    pad_src = np.zeros((GT, CAP), np.int16)
    pad_rel = np.zeros((GT, CAP), F)
    pad_w = np.zeros((GT, CAP), F)
    pad_src[gt, slot] = (srcs - (wid << 15)).astype(np.int16)
    pad_rel[gt, slot] = (dsts - (gt << 7)).astype(F)
    pad_w[gt, slot] = ws

    # per-tile meta layout: rel/w [GT,P,KcE] with entry (t,p,q)=edge (t, q*128+p)
    rel_all = pad_rel.reshape(GT, KcE, P).transpose(0, 2, 1)
    w_all = pad_w.reshape(GT, KcE, P).transpose(0, 2, 1)
    meta_all = np.concatenate([rel_all, w_all], axis=2)  # [GT,P,2KcE]

    xp = np.zeros((NPAD, P), F)
    xp[:N, 1:] = x
    xpT = xp.reshape(GT, P, P).transpose(0, 2, 1)  # [GT,P(feat),P(node)]

    ub1 = _host_ub(b1.astype(F), 1.0 / 3.0)
    ub2 = _host_ub(b2.astype(F), 0.5)
    ubl = _host_ub(bl.astype(F), 1.0)
    ubns = [float((u[1:] ** 2).sum(dtype=F)) for u in (ub1, ub2, ubl)]
    _build.ubns = ubns

    def ZWp(W, ub):
        We = W.astype(F).copy()
        We[:, 0] = 0
        wub = (W.astype(F).T @ ub.astype(F)).astype(F)
        wub[0] = 0
        return np.concatenate([We.T, wub[:, None]], axis=1)  # [in, out+1]

    DE = 129; DE3 = 65; OD = 64
    c32 = np.ascontiguousarray(ZWp(W1, ub1))  # [128,129] f32
    KcE_c = KcE
    c16 = np.zeros((P, DE + DE3 + 4 * P + OD + KcE_c * P), F16)
    o = 0
    c16[:, o:o + DE] = ZWp(W2, ub2).astype(F16); o += DE
    c16[:, o:o + DE3] = ZWp(Wl, ubl).astype(F16); o += DE3
    c16[:, o:o + P] = np.tile(np.arange(P, dtype=F16), (P, 1)); o += P
    c16[:, o:o + P] = np.eye(P, dtype=F16); o += P
    c16[:, o:o + P] = np.tile(ub1.astype(F16), (P, 1)); o += P
    c16[:, o:o + P] = np.tile(ub2.astype(F16), (P, 1)); o += P
    c16[:, o:o + OD] = np.tile(ubl.astype(F16), (P, 1)); o += OD
    c16[:, o:o + KcE_c * P] = np.tile(np.arange(P, dtype=F16), (P, KcE_c)); o += KcE_c * P

    def blk(a, inner):  # [T,P,inner] -> [NB,P,B*inner]
        return np.ascontiguousarray(
            a.reshape(NB, B, P, inner).transpose(0, 2, 1, 3).reshape(NB, P, B * inner))

    def wrap_idx(core_pad_src):  # [T, CAP] int16 -> [NB, 128, B*KcE*8]
        out = np.zeros((NB, P, B * KcE * 8), np.int16)
        for nb in range(NB):
            blk_rows = core_pad_src[nb * B:(nb + 1) * B]  # [B, CAP]
            off = 0
            for ww in range(NW):
                flat = blk_rows[:, col_base[ww]:col_base[ww + 1]].reshape(-1)
                wr = np.tile(flat.reshape(-1, 16).T, (8, 1))  # [128, n/16]
                out[nb, :, off:off + wr.shape[1]] = wr
                off += wr.shape[1]
        return out

    in_maps = []
    Tc = T
    for c in range(NC):
        sl = slice(c * Tc, (c + 1) * Tc)
        in_maps.append({
            "xpT": blk(xpT[sl], P).astype(F),
            "idx": wrap_idx(pad_src[sl]),
            "meta": blk(meta_all[sl], 2 * KcE).astype(F16),
            "c32": c32,
            "c16": c16,
        })
    return in_maps, T, tuple(Kcws), ubns


_CACHE = {}


def kernel(x, edge_index, edge_weight, W1, b1, W2, b2, Wl, bl, trace=False):
    N = x.shape[0]
    B = 7
    NPAD = ((N + NC * P - 1) // (NC * P)) * NC * P
    in_maps, T, Kcws, ubns = _prep(x, edge_index, edge_weight, W1, b1, W2, b2, Wl,
                                   bl, NPAD, B)
    key = (T, Kcws, NPAD, B, tuple(round(u, 9) for u in ubns))
    if key not in _CACHE:
        _CACHE[key] = _build(T, Kcws, NPAD, B)
    nc = _CACHE[key]
    r = run_bass_kernel_spmd(nc, in_maps, list(range(NC)), trace=trace)
    out = np.concatenate([r.results[c]["out"] for c in range(NC)], axis=0)[:N]
    kernel.last_exec_ns = r.exec_time_ns
    return out.astype(np.float32)


kernel.last_exec_ns = None


# revision 30
# speedup vs baseline: 1.0431x; 1.0431x over previous
"""HGCN forward on 8 TRN2 NeuronCores — restructured for throughput.

Strategy (graph/data parallel):
- Nodes padded to 100352 = 8*12544, sharded 12544/core (98 tiles of 128).
- Per-layer: matmul with an extra weight column (W^T u_b) so the mobius
  u_b dot-product falls out of the matmul; per-node scalar chains
  (expmap/mobius/logmap coefficients) are computed BATCHED on [128, 98]
  tiles once per phase instead of per-tile [128,1] ops.
- Gather table (logmap features) stored fp16: halves HBM gather traffic,
  AllGather bytes, and runs the segment-sum matmuls at fp16 PE rate.
- hyp_agg: edges sorted by destination tile; ONE batched indirect DMA
  gathers all chunks for a 7-tile block (amortizes the ~1us SWDGE fixed
  cost); one-hot*weight matrices built on DVE/Pool; dst-tile aggregates
  accumulate on the TensorEngine in PSUM.
"""
import os, sys, types
import numpy as np

sys.path.insert(0, "/opt/trn_rl_repo")

# NTFF profile hook shim (antenv.axon_hooks is absent in this image).
if "antenv.axon_hooks" not in sys.modules:
    _m = types.ModuleType("antenv.axon_hooks")
    _hh = [None]
    _m.set_axon_ntff_profile_hook = lambda h: _hh.__setitem__(0, h)
    _m.get_axon_ntff_profile_hook = lambda: _hh[0]
    sys.modules["antenv.axon_hooks"] = _m
    try:
        from trn_agent_boot.trn_boot import _ntff_profile_via_ctypes
        _m.set_axon_ntff_profile_hook(_ntff_profile_via_ctypes("/opt/axon/libaxon_pjrt.so"))
    except Exception:
        pass

import concourse.bass as bass
import concourse.tile as tile
from concourse import bacc, mybir
import concourse.bass_utils as _bu
_bu.upload_artifacts = lambda d: "local://skipped"
from concourse.bass_utils import run_bass_kernel_spmd
from concourse.library_config import mlp as _mlp_lib
from contextlib import ExitStack

F = np.float32
F16 = np.float16
EPS = 1e-7
MIN = 1e-15
NC = 8
P = 128
DT = mybir.dt.float32
DT16 = mybir.dt.float16
SKS = [float(np.sqrt(3.0)), float(np.sqrt(2.0)), 1.0]


def _host_ub(b, c):
    # u_b = logmap0(proj(expmap0(proj_tan0(b), c), c), c), faithful f32.
    K = F(1.0 / c)
    sK = F(np.sqrt(K))
    y = b[1:].astype(F)
    yn = max(np.sqrt((y * y).sum(dtype=F)), F(MIN))
    th = min(yn / sK, F(15.0))
    sh = F(np.sinh(th))
    ch = F(np.cosh(th))
    hb_s = sK * sh * y / yn
    hb0 = F(np.sqrt(max(K + (hb_s * hb_s).sum(dtype=F), F(EPS))))
    thh = max(hb0 / sK, F(1.0 + EPS))
    ac = F(np.log(thh + np.sqrt(thh * thh - 1)))
    ybn = max(F(np.sqrt((hb_s * hb_s).sum(dtype=F))), F(MIN))
    u_s = sK * ac * hb_s / ybn
    out = np.zeros(b.shape[0], F)
    out[1:] = u_s
    return out


def _build(T, Kcws, NPAD, B):
    """One SPMD program for all 8 cores. T tiles/core, Kcws chunks per
    src-window per tile, B tiles per gather/load block."""
    S = T * P
    NB = T // B
    KcE = int(sum(Kcws))
    WIN = 32768
    win_rows = [min(WIN, NPAD - w * WIN) for w in range(len(Kcws))]
    col_base = [0]
    for kc in Kcws:
        col_base.append(col_base[-1] + kc)
    IW = B * KcE * 8   # int16 idx cols per block
    DE = 129   # mv cols (128) + dub col
    DE3 = 65   # out_d (64) + dub col
    OD = 64
    nc = bacc.Bacc("TRN2", target_bir_lowering=False, debug=False, num_devices=NC)

    xpT_d = nc.dram_tensor("xpT", [NB, P, B * P], DT, kind="ExternalInput")
    idx_d = nc.dram_tensor("idx", [NB, P, IW], mybir.dt.int16, kind="ExternalInput")
    meta_d = nc.dram_tensor("meta", [NB, P, 2 * B * KcE], DT16, kind="ExternalInput")
    c32_d = nc.dram_tensor("c32", [P, DE], DT, kind="ExternalInput")
    c16_d = nc.dram_tensor("c16", [P, DE + DE3 + 4 * P + OD + KcE * P], DT16, kind="ExternalInput")
    out_d_t = nc.dram_tensor("out", [S, OD], DT, kind="ExternalOutput")

    xt1_sh = nc.dram_tensor("xt1_sh", [S, P], DT16)
    xt1_full = nc.dram_tensor("xt1_full", [NPAD, P], DT16, addr_space="Shared")
    xt2_sh = nc.dram_tensor("xt2_sh", [S, P], DT16)
    xt2_full = nc.dram_tensor("xt2_full", [NPAD, P], DT16, addr_space="Shared")

    A = mybir.AluOpType
    AF = mybir.ActivationFunctionType

    with tile.TileContext(nc) as tc, ExitStack() as ctx:
        cp = ctx.enter_context(tc.tile_pool(name="consts", bufs=1))
        xbp = ctx.enter_context(tc.tile_pool(name="xb", bufs=2))
        gp = ctx.enter_context(tc.tile_pool(name="gath", bufs=3))
        ibp = ctx.enter_context(tc.tile_pool(name="ib", bufs=2))
        mbp = ctx.enter_context(tc.tile_pool(name="mb", bufs=2))
        mtp = ctx.enter_context(tc.tile_pool(name="mt", bufs=2))
        rp = ctx.enter_context(tc.tile_pool(name="rp", bufs=3))
        rtp = ctx.enter_context(tc.tile_pool(name="rt", bufs=3))
        mvp = ctx.enter_context(tc.tile_pool(name="mv", bufs=1))
        btp = ctx.enter_context(tc.tile_pool(name="bt", bufs=1))
        stp = ctx.enter_context(tc.tile_pool(name="st", bufs=2))
        xop = ctx.enter_context(tc.tile_pool(name="xo", bufs=3))
        pag = ctx.enter_context(tc.tile_pool(name="pag", bufs=2, space="PSUM"))
        pmv = ctx.enter_context(tc.tile_pool(name="pmv", bufs=2, space="PSUM"))
        ptr = ctx.enter_context(tc.tile_pool(name="ptr", bufs=2, space="PSUM"))

        nc.gpsimd.load_library(_mlp_lib)
        c32 = cp.tile([P, DE], DT, name="c32t")
        nc.sync.dma_start(out=c32[:], in_=c32_d[:])
        c16 = cp.tile([P, DE + DE3 + 4 * P + OD + KcE * P], DT16, name="c16t")
        nc.sync.dma_start(out=c16[:], in_=c16_d[:])
        W1p = c32[:, 0:DE]
        o = 0
        W2p = c16[:, o:o + DE]; o += DE
        Wlp = c16[:, o:o + DE3]; o += DE3
        IOTA = c16[:, o:o + P]; o += P
        IDN = c16[:, o:o + P]; o += P
        UB1S = c16[:, o:o + P]; o += P
        UB2S = c16[:, o:o + P]; o += P
        UBLS = c16[:, o:o + OD]; o += OD
        IOTAW = c16[:, o:o + KcE * P].rearrange("p (c e) -> p c e", e=P); o += KcE * P

        # UBN2 constants are provided via closure at prep time
        UBNS = _build.ubns

        def bt_(nm):
            return btp.tile([P, T], DT, tag=nm, name=nm)

        def ts(o_, i_, s1, s2, op0, op1=None, eng=None):
            e = eng or nc.vector
            if op1 is None:
                e.tensor_scalar(o_[:], i_[:], s1, None, op0)
            else:
                e.tensor_scalar(o_[:], i_[:], s1, s2, op0, op1)

        def chain(MN2ap, DUBap, k_in, UBN2, AN2t=None, Y42t=None, k_ai=None,
                  k_ao=None, final=False):
            sk = SKS[k_in]; ik = 1.0 / sk; K = sk * sk
            if AN2t is not None:
                ski = SKS[k_ai]; iki = 1.0 / ski
                sko = SKS[k_ao]; iko = 1.0 / sko
                anr = bt_("anr"); nc.scalar.sqrt(anr[:], AN2t[:])
                anc = bt_("anc"); ts(anc, anr, MIN, None, A.max)
                th3 = bt_("th3"); ts(th3, anc, iki, 15.0, A.mult, A.min)
                ran = bt_("ran"); nc.vector.reciprocal(ran[:], anc[:])
                h3a = bt_("h3a"); nc.vector.tensor_tensor(h3a[:], th3[:], ran[:], A.mult)
                h3 = bt_("h3"); ts(h3, h3a, ski, None, A.mult)
                h3q = bt_("h3q"); nc.vector.tensor_tensor(h3q[:], h3[:], h3[:], A.mult)
                y42 = bt_("y42"); nc.vector.tensor_tensor(y42[:], Y42t[:], h3q[:], A.mult)
                y4r = bt_("y4r"); nc.scalar.sqrt(y4r[:], y42[:])
                y4c = bt_("y4c"); ts(y4c, y4r, MIN, None, A.max)
                th4 = bt_("th4"); ts(th4, y4c, iko, 15.0, A.mult, A.min)
                r4 = bt_("r4"); nc.vector.reciprocal(r4[:], y4c[:])
                m5a = bt_("m5a"); nc.vector.tensor_tensor(m5a[:], th4[:], r4[:], A.mult)
                m5 = bt_("m5"); ts(m5, m5a, sko, None, A.mult)
                sin = bt_("sin"); nc.vector.tensor_tensor(sin[:], h3[:], m5[:], A.mult)
                sin2 = bt_("sin2"); nc.vector.tensor_tensor(sin2[:], sin[:], sin[:], A.mult)
                mn2 = bt_("mn2"); nc.vector.tensor_tensor(mn2[:], MN2ap[:], sin2[:], A.mult)
                dub = bt_("dub"); nc.vector.tensor_tensor(dub[:], DUBap, sin[:], A.mult)
                mn2ap = mn2[:]; dub_ap = dub[:]
            else:
                sin = None
                mn2ap = MN2ap[:]; dub_ap = DUBap
            mnr = bt_("mnr"); nc.scalar.sqrt(mnr[:], mn2ap)
            mnc = bt_("mnc"); ts(mnc, mnr, MIN, None, A.max)
            thc = bt_("thc"); ts(thc, mnc, ik, 15.0, A.mult, A.min)
            ea = bt_("ea"); nc.scalar.activation(ea[:], thc[:], AF.Exp)
            eb = bt_("eb"); nc.scalar.activation(eb[:], thc[:], AF.Exp, scale=-1.0)
            sh2 = bt_("sh2"); nc.vector.tensor_tensor(sh2[:], ea[:], eb[:], A.subtract)
            ch2 = bt_("ch2"); nc.vector.tensor_tensor(ch2[:], ea[:], eb[:], A.add)
            rmn = bt_("rmn"); nc.vector.reciprocal(rmn[:], mnc[:])
            g1a = bt_("g1a"); nc.vector.tensor_tensor(g1a[:], sh2[:], rmn[:], A.mult)
            g1 = bt_("g1"); ts(g1, g1a, 0.5 * sk, None, A.mult)
            x0v = bt_("x0v"); ts(x0v, ch2, 0.5 * sk, None, A.mult)
            yna = bt_("yna"); nc.vector.tensor_tensor(yna[:], g1[:], mnc[:], A.mult)
            yn = bt_("yn"); ts(yn, yna, MIN, None, A.max)
            ryn = bt_("ryn"); nc.vector.reciprocal(ryn[:], yn[:])
            d1 = bt_("d1"); nc.vector.tensor_tensor(d1[:], g1[:], dub_ap, A.mult)
            ala = bt_("ala"); nc.vector.tensor_tensor(ala[:], d1[:], ryn[:], A.mult)
            alpha = bt_("alpha"); ts(alpha, ala, ik, None, A.mult)
            skx = bt_("skx"); ts(skx, x0v, sk, -1.0, A.subtract, A.mult)
            t2 = bt_("t2"); nc.vector.tensor_tensor(t2[:], alpha[:], skx[:], A.mult)
            scal1 = bt_("scal1"); nc.vector.tensor_tensor(scal1[:], t2[:], ryn[:], A.mult)
            gg = bt_("gg"); nc.vector.tensor_tensor(gg[:], scal1[:], g1[:], A.mult)
            gm = bt_("gm"); nc.vector.tensor_tensor(gm[:], g1[:], mn2ap, A.mult)
            u1 = bt_("u1"); nc.vector.tensor_tensor(u1[:], gg[:], gm[:], A.mult)
            ux = bt_("ux"); nc.vector.tensor_tensor(ux[:], d1[:], u1[:], A.subtract)
            rx0 = bt_("rx0"); nc.vector.reciprocal(rx0[:], x0v[:])
            v0 = bt_("v0"); nc.vector.tensor_tensor(v0[:], ux[:], rx0[:], A.mult)
            w1 = bt_("w1"); nc.vector.tensor_tensor(w1[:], gg[:], mn2ap, A.mult)
            # w3 = 2*dub - w1
            d2 = bt_("d2"); nc.vector.tensor_scalar(d2[:], dub_ap, 2.0, None, A.mult)
            w3 = bt_("w3"); nc.vector.tensor_tensor(w3[:], d2[:], w1[:], A.subtract)
            mdpa = bt_("mdpa"); nc.vector.tensor_tensor(mdpa[:], gg[:], w3[:], A.mult)
            mdp = bt_("mdp"); ts(mdp, mdpa, -1.0, UBN2, A.mult, A.add)
            v0q = bt_("v0q"); nc.vector.tensor_tensor(v0q[:], v0[:], v0[:], A.mult)
            md = bt_("md"); nc.vector.tensor_tensor(md[:], mdp[:], v0q[:], A.subtract)
            mdc = bt_("mdc"); ts(mdc, md, EPS, None, A.max)
            nur = bt_("nur"); nc.scalar.sqrt(nur[:], mdc[:])
            th2 = bt_("th2"); ts(th2, nur, 1e6, ik, A.min, A.mult)
            th2m = bt_("th2m"); ts(th2m, th2, MIN, None, A.max)
            th2c = bt_("th2c"); ts(th2c, th2m, 15.0, None, A.min)
            ea2 = bt_("ea2"); nc.scalar.activation(ea2[:], th2c[:], AF.Exp)
            eb2 = bt_("eb2"); nc.scalar.activation(eb2[:], th2c[:], AF.Exp, scale=-1.0)
            sh22 = bt_("sh22"); nc.vector.tensor_tensor(sh22[:], ea2[:], eb2[:], A.subtract)
            ch22 = bt_("ch22"); nc.vector.tensor_tensor(ch22[:], ea2[:], eb2[:], A.add)
            rt2 = bt_("rt2"); nc.vector.reciprocal(rt2[:], th2m[:])
            s2a = bt_("s2a"); nc.vector.tensor_tensor(s2a[:], sh22[:], rt2[:], A.mult)
            s2 = bt_("s2"); ts(s2, s2a, 0.5, None, A.mult)
            ss = bt_("ss"); nc.vector.tensor_tensor(ss[:], s2[:], scal1[:], A.mult)
            cc = bt_("cc"); ts(cc, ch22, 0.5, None, A.mult)
            Aa = bt_("Aa"); nc.vector.tensor_tensor(Aa[:], cc[:], ss[:], A.subtract)
            Av = bt_("Av"); nc.vector.tensor_tensor(Av[:], Aa[:], g1[:], A.mult)
            A2 = bt_("A2"); nc.vector.tensor_tensor(A2[:], Av[:], Av[:], A.mult)
            tA = bt_("tA"); nc.vector.tensor_tensor(tA[:], A2[:], mn2ap, A.mult)
            ABv = bt_("ABv"); nc.vector.tensor_tensor(ABv[:], Av[:], s2[:], A.mult)
            tABa = bt_("tABa")
            nc.vector.tensor_tensor(tABa[:], ABv[:], dub_ap, A.mult)
            tAB = bt_("tAB"); ts(tAB, tABa, 2.0, None, A.mult)
            tBa = bt_("tBa"); nc.vector.tensor_tensor(tBa[:], s2[:], s2[:], A.mult)
            tB = bt_("tB"); ts(tB, tBa, UBN2, None, A.mult)
            l2a = bt_("l2a"); nc.vector.tensor_tensor(l2a[:], tA[:], tAB[:], A.add)
            ln2 = bt_("ln2"); nc.vector.tensor_tensor(ln2[:], l2a[:], tB[:], A.add)
            lnk = bt_("lnk"); ts(lnk, ln2, K, None, A.add)
            L0 = bt_("L0"); nc.scalar.sqrt(L0[:], lnk[:])
            if final:
                SA = bt_("SA")
                nc.vector.tensor_tensor(SA[:], Av[:], sin[:], A.mult)
                return SA, s2, L0
            thL = bt_("thL"); ts(thL, L0, ik, 1.0 + EPS, A.mult, A.max)
            tq = bt_("tq"); nc.vector.tensor_tensor(tq[:], thL[:], thL[:], A.mult)
            tqm = bt_("tqm"); ts(tqm, tq, -1.0, None, A.add)
            sq = bt_("sq"); nc.scalar.sqrt(sq[:], tqm[:])
            ai = bt_("ai"); nc.vector.tensor_tensor(ai[:], thL[:], sq[:], A.add)
            ac = bt_("ac"); nc.scalar.activation(ac[:], ai[:], AF.Ln)
            yr2 = bt_("yr2"); nc.scalar.sqrt(yr2[:], ln2[:])
            yc2 = bt_("yc2"); ts(yc2, yr2, MIN, None, A.max)
            ry2 = bt_("ry2"); nc.vector.reciprocal(ry2[:], yc2[:])
            fLa = bt_("fLa"); nc.vector.tensor_tensor(fLa[:], ac[:], ry2[:], A.mult)
            fL = bt_("fL"); ts(fL, fLa, sk, None, A.mult)
            sAa = bt_("sAa"); nc.vector.tensor_tensor(sAa[:], fL[:], Av[:], A.mult)
            SA = bt_("SA")
            if sin is not None:
                nc.vector.tensor_tensor(SA[:], sAa[:], sin[:], A.mult)
            else:
                nc.vector.tensor_copy(SA[:], sAa[:])
            SB = bt_("SB"); nc.vector.tensor_tensor(SB[:], fL[:], s2[:], A.mult)
            return SA, SB

        # ================= Phase A =================
        MN2 = btp.tile([P, T], DT, tag="MN2", name="MN2a")
        MV = mvp.tile([P, T * DE], DT, tag="MV", name="MVa")
        for nb in range(NB):
            xblk = xbp.tile([P, B * P], DT, name="xblk")
            nc.sync.dma_start(out=xblk[:], in_=xpT_d[nb])
            for b in range(B):
                t = nb * B + b
                mv = pmv.tile([P, DE], DT, space="PSUM", tag="mv", name="mvp")
                nc.tensor.matmul(mv[:], lhsT=xblk[:, b * P:(b + 1) * P],
                                 rhs=W1p, start=True, stop=True)
                scr = stp.tile([P, P - 1], DT, tag="s32", name="scr")
                nc.scalar.activation(scr[:], mv[:, 1:P], AF.Square,
                                     accum_out=MN2[:, t:t + 1])
                nc.scalar.copy(MV[:, t * DE:(t + 1) * DE], mv[:])
        dubA = MV[:].rearrange("p (t d) -> p t d", d=DE)[:, :, DE - 1:DE].squeeze(2)
        SA, SB = chain(MN2, dubA, 0, UBNS[0])
        for t in range(T):
            xa = xop.tile([P, P], DT16, tag="xa", name="xa")
            nc.scalar.activation(xa[:], MV[:, t * DE:t * DE + P], AF.Copy,
                                 scale=SA[:, t:t + 1])
            ub = xop.tile([P, P], DT16, tag="ub", name="ub")
            nc.vector.tensor_scalar(ub[:], UB1S, SB[:, t:t + 1], None, A.mult)
            xt = xop.tile([P, P], DT16, tag="xt", name="xt")
            nc.vector.tensor_tensor(xt[:], xa[:], ub[:], A.add)
            nc.sync.dma_start(out=xt1_sh[t * P:(t + 1) * P, :], in_=xt[:])
        nc.gpsimd.collective_compute("AllGather", A.bypass,
                                     replica_groups=[list(range(NC))],
                                     ins=[xt1_sh[:]], outs=[xt1_full[:]])

        # ============ Phases B & C (agg + linear) ============
        def agg_phase(table, Wp, DEx, k_ai, k_ao, UBN2, UBS, final, sh_out, full_out):
            MN2 = btp.tile([P, T], DT, tag="MN2", name="MN2x")
            AN2 = btp.tile([P, T], DT, tag="AN2", name="AN2x")
            Y42 = btp.tile([P, T], DT, tag="Y42", name="Y42x")
            MV = mvp.tile([P, T * DE], DT, tag="MV", name="MVx")
            for nb in range(NB):
                idxb = ibp.tile([P, IW], mybir.dt.int16, name="idxb")
                nc.sync.dma_start(out=idxb[:], in_=idx_d[nb])
                metb = mbp.tile([P, 2 * B * KcE], DT16, name="metb")
                nc.sync.dma_start(out=metb[:], in_=meta_d[nb])
                G = gp.tile([P, B * KcE * P], DT16, tag="G", name="G")
                off_c = 0
                off_i = 0
                for w, kc in enumerate(Kcws):
                    n_w = B * kc * P
                    nc.gpsimd.dma_gather(
                        out_ap=G[:, off_c:off_c + n_w].rearrange(
                            "p (c e) -> p c e", e=P),
                        in_ap=table[w * WIN:w * WIN + win_rows[w], :],
                        idxs_ap=idxb[:, off_i:off_i + n_w // 16],
                        num_idxs=n_w, num_idxs_reg=n_w, elem_size=P,
                        single_packet=False)
                    off_c += n_w
                    off_i += n_w // 16
                for b in range(B):
                    t = nb * B + b
                    rels = metb[:, b * 2 * KcE:b * 2 * KcE + KcE]
                    wcol = metb[:, b * 2 * KcE + KcE:b * 2 * KcE + 2 * KcE]
                    MtA = mtp.tile([P, KcE * P], DT16, tag="MtA", name="MtA")
                    nc.vector.tensor_tensor(
                        MtA[:].rearrange("p (c e) -> p c e", e=P), IOTAW,
                        rels.unsqueeze(2).broadcast_to([P, KcE, P]), A.is_equal)
                    MtW = mtp.tile([P, KcE * P], DT16, tag="MtW", name="MtW")
                    nc.vector.tensor_tensor(
                        MtW[:].rearrange("p (c e) -> p c e", e=P),
                        MtA[:].rearrange("p (c e) -> p c e", e=P),
                        wcol.unsqueeze(2).broadcast_to([P, KcE, P]), A.mult)
                    agg = pag.tile([P, P], DT, space="PSUM", tag="agg", name="aggp")
                    for q in range(KcE):
                        # window of chunk q and its column in G
                        w = 0
                        while q >= col_base[w + 1]:
                            w += 1
                        j = q - col_base[w]
                        col = (B * col_base[w] + b * Kcws[w] + j) * P
                        nc.tensor.matmul(agg[:], lhsT=MtW[:, q * P:(q + 1) * P],
                                         rhs=G[:, col:col + P],
                                         start=(q == 0), stop=(q == KcE - 1))
                    s32a = stp.tile([P, P - 1], DT, tag="s32a", name="s32a")
                    nc.scalar.activation(s32a[:], agg[:, 1:P], AF.Square,
                                         accum_out=AN2[:, t:t + 1])
                    R = rp.tile([P, P], DT16, tag="R", name="R")
                    nc.scalar.activation(R[:], agg[:], AF.Relu)
                    s16b = stp.tile([P, P - 1], DT16, tag="s16b", name="s16b")
                    nc.scalar.activation(s16b[:], R[:, 1:P], AF.Square,
                                         accum_out=Y42[:, t:t + 1])
                    trp = ptr.tile([P, P], DT16, space="PSUM", tag="trp", name="trp")
                    nc.tensor.transpose(trp[:], R[:], IDN)
                    rT = rtp.tile([P, P], DT16, tag="rT", name="rT")
                    nc.scalar.copy(rT[:], trp[:])
                    mv = pmv.tile([P, DEx], DT, space="PSUM", tag="mv", name="mvp")
                    nc.tensor.matmul(mv[:], lhsT=rT[:], rhs=Wp, start=True, stop=True)
                    scr = stp.tile([P, P - 1], DT, tag="s32", name="scr")
                    nc.scalar.activation(scr[:, 0:DEx - 2], mv[:, 1:DEx - 1], AF.Square,
                                         accum_out=MN2[:, t:t + 1])
                    nc.scalar.copy(MV[:, t * DE:t * DE + DEx], mv[:])
            dubv = MV[:].rearrange("p (t d) -> p t d", d=DE)[:, :, DEx - 1:DEx].squeeze(2)
            if final:
                SA3, SB3, L0 = chain(MN2, dubv, k_ao, UBN2, AN2, Y42, k_ai, k_ao,
                                     final=True)
                for t in range(T):
                    o1 = xop.tile([P, OD - 1], DT, tag="o1", name="o1")
                    nc.scalar.activation(o1[:], MV[:, t * DE + 1:t * DE + OD], AF.Copy,
                                         scale=SA3[:, t:t + 1])
                    u3 = xop.tile([P, OD - 1], DT, tag="u3", name="u3")
                    nc.vector.tensor_scalar(u3[:], UBS[:, 1:OD], SB3[:, t:t + 1],
                                            None, A.mult)
                    ot = xop.tile([P, OD], DT, tag="ot", name="ot")
                    nc.vector.tensor_tensor(ot[:, 1:OD], o1[:], u3[:], A.add)
                    nc.vector.tensor_copy(ot[:, 0:1], L0[:, t:t + 1])
                    nc.sync.dma_start(out=out_d_t[t * P:(t + 1) * P, :], in_=ot[:])
            else:
                SA, SB = chain(MN2, dubv, k_ao, UBN2, AN2, Y42, k_ai, k_ao)
                for t in range(T):
                    xa = xop.tile([P, P], DT16, tag="xa", name="xa")
                    nc.scalar.activation(xa[:], MV[:, t * DE:t * DE + P], AF.Copy,
                                         scale=SA[:, t:t + 1])
                    ub = xop.tile([P, P], DT16, tag="ub", name="ub")
                    nc.vector.tensor_scalar(ub[:], UBS, SB[:, t:t + 1], None, A.mult)
                    xt = xop.tile([P, P], DT16, tag="xt", name="xt")
                    nc.vector.tensor_tensor(xt[:], xa[:], ub[:], A.add)
                    nc.sync.dma_start(out=sh_out[t * P:(t + 1) * P, :], in_=xt[:])
                nc.gpsimd.collective_compute("AllGather", A.bypass,
                                             replica_groups=[list(range(NC))],
                                             ins=[sh_out[:]], outs=[full_out[:]])

        stages = int(os.environ.get("KSTAGES", "3"))
        if stages >= 2:
            agg_phase(xt1_full, W2p, DE, 0, 1, UBNS[1], UB2S, False, xt2_sh, xt2_full)
        if stages >= 3:
            agg_phase(xt2_full, Wlp, DE3, 1, 2, UBNS[2], UBLS, True, None, None)
        if stages < 3:
            z = xop.tile([P, OD], DT, tag="z", name="z")
            nc.vector.memset(z[:], 0.0)
            for t in range(T):
                nc.sync.dma_start(out=out_d_t[t * P:(t + 1) * P, :], in_=z[:])

    nc.compile()
    return nc


def _prep(x, edge_index, edge_weight, W1, b1, W2, b2, Wl, bl, NPAD, B):
    N = x.shape[0]
    S = NPAD // NC
    T = S // P
    GT = NPAD // P
    NB = T // B
    WIN = 32768
    NW = (NPAD + WIN - 1) // WIN
    src = edge_index[0].astype(np.int64)
    dst = edge_index[1].astype(np.int64)
    w = edge_weight.astype(F)
    gt_u = dst >> 7
    wid_u = src >> 15
    # group edges by (dst tile, src window)
    order = np.lexsort((wid_u, gt_u))
    srcs, dsts, ws = src[order], dst[order], w[order]
    gt, wid = gt_u[order], wid_u[order]
    cnt_tw = np.zeros((GT, NW), np.int64)
    np.add.at(cnt_tw, (gt, wid), 1)
    Kcws = [max(1, int(np.ceil(cnt_tw[:, ww].max() / P))) for ww in range(NW)]
    KcE = int(sum(Kcws))
    CAPS = [kc * P for kc in Kcws]
    col_base = np.zeros(NW + 1, np.int64)
    col_base[1:] = np.cumsum(CAPS)
    CAP = int(col_base[-1])
    starts_tw = np.zeros(GT * NW, np.int64)
    starts_tw[1:] = np.cumsum(cnt_tw.reshape(-1))[:-1]
    starts_tw = starts_tw.reshape(GT, NW)
    pos = np.arange(len(srcs)) - starts_tw[gt, wid]
    slot = col_base[wid] + pos
    pad_src = np.zeros((GT, CAP), np.int16)
    pad_rel = np.zeros((GT, CAP), F)
    pad_w = np.zeros((GT, CAP), F)
    pad_src[gt, slot] = (srcs - (wid << 15)).astype(np.int16)
    pad_rel[gt, slot] = (dsts - (gt << 7)).astype(F)
    pad_w[gt, slot] = ws

    # per-tile meta layout: rel/w [GT,P,KcE] with entry (t,p,q)=edge (t, q*128+p)
    rel_all = pad_rel.reshape(GT, KcE, P).transpose(0, 2, 1)
    w_all = pad_w.reshape(GT, KcE, P).transpose(0, 2, 1)
    meta_all = np.concatenate([rel_all, w_all], axis=2)  # [GT,P,2KcE]

    xp = np.zeros((NPAD, P), F)
    xp[:N, 1:] = x
    xpT = xp.reshape(GT, P, P).transpose(0, 2, 1)  # [GT,P(feat),P(node)]

    ub1 = _host_ub(b1.astype(F), 1.0 / 3.0)
    ub2 = _host_ub(b2.astype(F), 0.5)
    ubl = _host_ub(bl.astype(F), 1.0)
    ubns = [float((u[1:] ** 2).sum(dtype=F)) for u in (ub1, ub2, ubl)]
    _build.ubns = ubns

    def ZWp(W, ub):
        We = W.astype(F).copy()
        We[:, 0] = 0
        wub = (W.astype(F).T @ ub.astype(F)).astype(F)
        wub[0] = 0
        return np.concatenate([We.T, wub[:, None]], axis=1)  # [in, out+1]

    DE = 129; DE3 = 65; OD = 64
    c32 = np.ascontiguousarray(ZWp(W1, ub1))  # [128,129] f32
    KcE_c = KcE
    c16 = np.zeros((P, DE + DE3 + 4 * P + OD + KcE_c * P), F16)
    o = 0
    c16[:, o:o + DE] = ZWp(W2, ub2).astype(F16); o += DE
    c16[:, o:o + DE3] = ZWp(Wl, ubl).astype(F16); o += DE3
    c16[:, o:o + P] = np.tile(np.arange(P, dtype=F16), (P, 1)); o += P
    c16[:, o:o + P] = np.eye(P, dtype=F16); o += P
    c16[:, o:o + P] = np.tile(ub1.astype(F16), (P, 1)); o += P
    c16[:, o:o + P] = np.tile(ub2.astype(F16), (P, 1)); o += P
    c16[:, o:o + OD] = np.tile(ubl.astype(F16), (P, 1)); o += OD
    c16[:, o:o + KcE_c * P] = np.tile(np.arange(P, dtype=F16), (P, KcE_c)); o += KcE_c * P

    def blk(a, inner):  # [T,P,inner] -> [NB,P,B*inner]
        return np.ascontiguousarray(
            a.reshape(NB, B, P, inner).transpose(0, 2, 1, 3).reshape(NB, P, B * inner))

    def wrap_idx(core_pad_src):  # [T, CAP] int16 -> [NB, 128, B*KcE*8]
        out = np.zeros((NB, P, B * KcE * 8), np.int16)
        for nb in range(NB):
            blk_rows = core_pad_src[nb * B:(nb + 1) * B]  # [B, CAP]
            off = 0
            for ww in range(NW):
                flat = blk_rows[:, col_base[ww]:col_base[ww + 1]].reshape(-1)
                wr = np.tile(flat.reshape(-1, 16).T, (8, 1))  # [128, n/16]
                out[nb, :, off:off + wr.shape[1]] = wr
                off += wr.shape[1]
        return out

    in_maps = []
    Tc = T
    for c in range(NC):
        sl = slice(c * Tc, (c + 1) * Tc)
        in_maps.append({
            "xpT": blk(xpT[sl], P).astype(F),
            "idx": wrap_idx(pad_src[sl]),
            "meta": blk(meta_all[sl], 2 * KcE).astype(F16),
            "c32": c32,
            "c16": c16,
        })
    return in_maps, T, tuple(Kcws), ubns


_CACHE = {}


def kernel(x, edge_index, edge_weight, W1, b1, W2, b2, Wl, bl, trace=False):
    N = x.shape[0]
    B = 7
    NPAD = ((N + NC * P - 1) // (NC * P)) * NC * P
    in_maps, T, Kcws, ubns = _prep(x, edge_index, edge_weight, W1, b1, W2, b2, Wl,
                                   bl, NPAD, B)
    key = (T, Kcws, NPAD, B, tuple(round(u, 9) for u in ubns))
    if key not in _CACHE:
        _CACHE[key] = _build(T, Kcws, NPAD, B)
    nc = _CACHE[key]
    r = run_bass_kernel_spmd(nc, in_maps, list(range(NC)), trace=trace)
    out = np.concatenate([r.results[c]["out"] for c in range(NC)], axis=0)[:N]
    kernel.last_exec_ns = r.exec_time_ns
    return out.astype(np.float32)


kernel.last_exec_ns = None
